# revision 1
# baseline (speedup 1.0000x reference)
"""Trainium2 Bass kernel for nn_CAN_Layer_74775380623980.

Math: with sequence length L=1, softmax over the single key is exactly 1.0
and the reference's masks are overwritten with ones, so the whole cross
attention collapses to

    E   = (protein @ Wv_p + drug @ Wv_d) / 2          # [N, 2048]
    out = concat([E, E], axis=1)                      # [N, 4096]

Sharding: pure data parallel, batch N=16384 split 8 ways (2048 rows/core);
the two V projection weights are replicated. Per core the device computes
E_shard = Xp @ (Wv_p/2) + Xd @ (Wv_d/2) as fp16 matmuls (fp32 PSUM
accumulation). The 0.5 scale is folded into the weights on the host (exact,
power of two). Activations are pre-transposed/tiled on the host so every DMA
is contiguous 4KB-per-partition and the PE runs K-contiguous back-to-back
matmuls with no on-device transposes.
"""

import numpy as np

P = 128          # partitions / systolic tile
N_FULL = 16384
D = 2048         # contraction dim
HID = 2048       # output dim per projection
NCORES = 8
M_SH = N_FULL // NCORES   # 2048 rows per core
KT = D // P               # 16 k-tiles
NBLK = 512                # matmul free dim (one PSUM bank of fp32)
NB = HID // NBLK          # 4 n-blocks
MT_FULL = M_SH // P       # 16 m-tiles


def _build_module(mt_tiles=MT_FULL, reps=1, xbufs=2, obufs=2, paired=True):
    """reps>1 wraps the m-loop in a device-side For_i — used only for
    wall-clock benchmarking (amplifies device time above RPC noise)."""
    import concourse.bass as bass  # noqa: F401
    import concourse.mybir as mybir
    import concourse.tile as tile
    from concourse import bacc

    fp16 = mybir.dt.float16
    f32 = mybir.dt.float32

    nc = bacc.Bacc("TRN2", target_bir_lowering=False, debug=False)

    xp_h = nc.dram_tensor("xp", [mt_tiles, P, KT, P], fp16, kind="ExternalInput")
    xd_h = nc.dram_tensor("xd", [mt_tiles, P, KT, P], fp16, kind="ExternalInput")
    wp_h = nc.dram_tensor("wp", [KT, P, HID], fp16, kind="ExternalInput")
    wd_h = nc.dram_tensor("wd", [KT, P, HID], fp16, kind="ExternalInput")
    out_h = nc.dram_tensor("out", [mt_tiles * P, HID], f32, kind="ExternalOutput")

    with tile.TileContext(nc) as tc:
        with (
            tc.tile_pool(name="wpool", bufs=1) as wpool,
            tc.tile_pool(name="xpool", bufs=(2 * xbufs if paired else xbufs)) as xpool,
            tc.tile_pool(name="opool", bufs=obufs) as opool,
            tc.tile_pool(name="psum", bufs=(1 if paired else 2), space="PSUM") as pp,
        ):
            x_tiles = {}

            def load_x(mt):
                tp = xpool.tile([P, KT, P], fp16, tag="xp", name=f"xp_{mt}")
                nc.sync.dma_start(tp[:], xp_h[mt])
                td = xpool.tile([P, KT, P], fp16, tag="xd", name=f"xd_{mt}")
                nc.sync.dma_start(td[:], xd_h[mt])
                x_tiles[mt] = (tp, td)

            wp_sb, wd_sb = [], []

            def load_weights():
                wp_sb.clear()
                wd_sb.clear()
                for j in range(KT):
                    tw = wpool.tile([P, HID], fp16, tag=f"wp{j}", name=f"wp_{j}")
                    nc.sync.dma_start(tw[:], wp_h[j])
                    wp_sb.append(tw)
                    tw = wpool.tile([P, HID], fp16, tag=f"wd{j}", name=f"wd_{j}")
                    nc.sync.dma_start(tw[:], wd_h[j])
                    wd_sb.append(tw)

            next_load = [1]  # load_x(0) is issued before the loop

            def ensure_loads(upto):
                while next_load[0] <= min(upto, mt_tiles - 1):
                    load_x(next_load[0])
                    next_load[0] += 1

            def m_loop():
                next_load[0] = 1
                for mt in range(mt_tiles):
                    ensure_loads(mt + xbufs - 1)
                    xp_t, xd_t = x_tiles.pop(mt)
                    psums = [
                        pp.tile([P, NBLK], f32, tag=f"ps{nb}", name=f"ps_{mt}_{nb}")
                        for nb in range(NB)
                    ]
                    for j in range(KT):
                        for nb in range(NB):
                            nc.tensor.matmul(
                                psums[nb][:],
                                xp_t[:, j, :],
                                wp_sb[j][:, nb * NBLK : (nb + 1) * NBLK],
                                start=(j == 0),
                                stop=False,
                            )
                        for nb in range(NB):
                            nc.tensor.matmul(
                                psums[nb][:],
                                xd_t[:, j, :],
                                wd_sb[j][:, nb * NBLK : (nb + 1) * NBLK],
                                start=False,
                                stop=(j == KT - 1),
                            )
                    out_t = opool.tile([P, HID], f32, tag="out", name=f"out_{mt}")
                    for nb in range(NB):
                        nc.vector.tensor_copy(
                            out_t[:, nb * NBLK : (nb + 1) * NBLK], psums[nb][:]
                        )
                    nc.sync.dma_start(out_h[mt * P : (mt + 1) * P, :], out_t[:])

            def m_loop_paired():
                # two m-tiles in flight per j-step (8 PSUM banks): during the
                # cold-start weight stream-in, the PE consumes each arriving
                # k-strip with 2x the matmul work, eliminating starvation
                assert mt_tiles % 2 == 0, "paired m-loop needs an even m-tile count"
                next_load[0] = 1
                for mt0 in range(0, mt_tiles, 2):
                    pair = [mt0, mt0 + 1]
                    ensure_loads(mt0 + 1 + 2 * (xbufs - 1))
                    xt = {mt: x_tiles.pop(mt) for mt in pair}
                    psums = {
                        (h, nb): pp.tile(
                            [P, NBLK], f32, tag=f"ps{h}_{nb}", name=f"ps_{mt0}_{h}_{nb}"
                        )
                        for h in range(2)
                        for nb in range(NB)
                    }
                    for j in range(KT):
                        for h, mt in enumerate(pair):
                            for nb in range(NB):
                                nc.tensor.matmul(
                                    psums[h, nb][:],
                                    xt[mt][0][:, j, :],
                                    wp_sb[j][:, nb * NBLK : (nb + 1) * NBLK],
                                    start=(j == 0),
                                    stop=False,
                                )
                        for h, mt in enumerate(pair):
                            for nb in range(NB):
                                nc.tensor.matmul(
                                    psums[h, nb][:],
                                    xt[mt][1][:, j, :],
                                    wd_sb[j][:, nb * NBLK : (nb + 1) * NBLK],
                                    start=False,
                                    stop=(j == KT - 1),
                                )
                    for h, mt in enumerate(pair):
                        out_t = opool.tile([P, HID], f32, tag="out", name=f"out_{mt}")
                        for nb in range(NB):
                            nc.vector.tensor_copy(
                                out_t[:, nb * NBLK : (nb + 1) * NBLK], psums[h, nb][:]
                            )
                        nc.sync.dma_start(out_h[mt * P : (mt + 1) * P, :], out_t[:])

            body = m_loop_paired if paired else m_loop
            if reps == 1:
                # first m-tile's activations go out before the weight preload
                # so the PE starts as soon as strip j=0 of the weights lands
                load_x(0)
                load_weights()
                body()
            else:
                # full body (weight preload included) repeats: per-rep wall
                # time == one-shot kernel exec time
                with tc.For_i(0, reps, 1):
                    load_x(0)
                    load_weights()
                    body()

    nc.compile()
    return nc


def _prep_inputs(protein, drug, Wv_p, Wv_d, mt_tiles=MT_FULL):
    """Host-side shard + transpose-tile + fp16 cast."""
    wp = (0.5 * np.asarray(Wv_p, dtype=np.float32)).astype(np.float16)
    wd = (0.5 * np.asarray(Wv_d, dtype=np.float32)).astype(np.float16)
    wp = np.ascontiguousarray(wp.reshape(KT, P, HID))
    wd = np.ascontiguousarray(wd.reshape(KT, P, HID))

    def tile_x(x):
        # [M_SH, D] -> [mt, p, j, m'] with x_t[mt, p, j, m'] = x[mt*P+m', j*P+p]
        t = x.reshape(mt_tiles, P, KT, P).transpose(0, 3, 2, 1)
        return np.ascontiguousarray(t.astype(np.float16))

    protein = np.asarray(protein, dtype=np.float32)
    drug = np.asarray(drug, dtype=np.float32)
    in_maps = []
    rows = mt_tiles * P
    for c in range(NCORES):
        sl = slice(c * M_SH, c * M_SH + rows)
        in_maps.append(
            {
                "xp": tile_x(protein[sl]),
                "xd": tile_x(drug[sl]),
                "wp": wp,
                "wd": wd,
            }
        )
    return in_maps


_MODULE_CACHE = {}


def _run(protein, drug, Wv_p, Wv_d, trace=False, mt_tiles=MT_FULL):
    from concourse.bass_utils import run_bass_kernel_spmd

    nc = _MODULE_CACHE.get(mt_tiles)
    if nc is None:
        nc = _MODULE_CACHE[mt_tiles] = _build_module(mt_tiles)
    in_maps = _prep_inputs(protein, drug, Wv_p, Wv_d, mt_tiles)
    res = run_bass_kernel_spmd(nc, in_maps, list(range(NCORES)), trace=trace)
    E = np.concatenate(
        [np.asarray(r["out"], dtype=np.float32) for r in res.results], axis=0
    )
    return E, res


def kernel(
    protein,
    drug,
    mask_prot=None,
    mask_drug=None,
    Wq_p=None,
    Wk_p=None,
    Wv_p=None,
    Wq_d=None,
    Wk_d=None,
    Wv_d=None,
):
    E, _ = _run(protein, drug, Wv_p, Wv_d, trace=False)
    return np.concatenate([E, E], axis=1)


def kernel_profiled(**inputs):
    E, res = _run(
        inputs["protein"], inputs["drug"], inputs["Wv_p"], inputs["Wv_d"], trace=False
    )
    out = np.concatenate([E, E], axis=1)
    return out, res



# revision 2
# speedup vs baseline: 1.0283x; 1.0283x over previous
"""Trainium2 Bass kernel for nn_CAN_Layer_74775380623980.

Math: with sequence length L=1, softmax over the single key is exactly 1.0
and the reference's masks are overwritten with ones, so the whole cross
attention collapses to

    E   = (protein @ Wv_p + drug @ Wv_d) / 2          # [N, 2048]
    out = concat([E, E], axis=1)                      # [N, 4096]

Sharding: pure data parallel, batch N=16384 split 8 ways (2048 rows/core);
the two V projection weights are replicated.

Precision/speed split: per tensor, K-strips 0..11 (1536 of 2048) run as
fp16 matmuls; strips 12..15 run as fp8-e4m3 DoubleRow matmuls (2 K-strips
per instruction at 2x PE rate). Both paths accumulate into the same PSUM
bank at a common scale of 2^16 (x scaled by 16, weights by 0.5*4096 — all
powers of two, exact), undone by a scaled PSUM->SBUF copy. Measured
end-to-end rel_fro error vs the fp32 reference: 1.9e-2 (< 2e-2 gate).
"""

import numpy as np
import ml_dtypes

P = 128          # partitions / systolic tile
N_FULL = 16384
D = 2048         # contraction dim per tensor
HID = 2048       # output dim per projection
NCORES = 8
M_SH = N_FULL // NCORES   # 2048 rows per core
KT = D // P               # 16 k-strips per tensor
J16 = 12                  # fp16 k-strips per tensor
DJ8 = (KT - J16) // 2     # fp8 DoubleRow steps per tensor (2 strips each)
NBLK = 512                # matmul free dim (one PSUM bank of fp32)
NB = HID // NBLK          # 4 n-blocks
MT_FULL = M_SH // P       # 16 m-tiles
XSCALE = 16.0             # x pre-scale (power of two)
WSCALE = 2048.0           # 0.5 (reference's /2) * 4096 weight pre-scale
OSCALE = 1.0 / (XSCALE * 4096.0)  # PSUM de-scale = 2^-16


def _build_module(mt_tiles=MT_FULL, reps=1, xbufs=2, obufs=2, paired=True):
    """reps>1 wraps the whole body in a device-side For_i — used only for
    wall-clock benchmarking (amplifies device time above RPC noise)."""
    import concourse.bass as bass  # noqa: F401
    import concourse.mybir as mybir
    import concourse.tile as tile
    from concourse import bacc

    fp16 = mybir.dt.float16
    fp8 = mybir.dt.float8e4
    f32 = mybir.dt.float32
    DR = mybir.MatmulPerfMode.DoubleRow

    nc = bacc.Bacc("TRN2", target_bir_lowering=False, debug=False)

    xp16_h = nc.dram_tensor("xp16", [mt_tiles, P, J16, P], fp16, kind="ExternalInput")
    xd16_h = nc.dram_tensor("xd16", [mt_tiles, P, J16, P], fp16, kind="ExternalInput")
    xp8_h = nc.dram_tensor("xp8", [mt_tiles, P, DJ8, 2, P], fp8, kind="ExternalInput")
    xd8_h = nc.dram_tensor("xd8", [mt_tiles, P, DJ8, 2, P], fp8, kind="ExternalInput")
    wp16_h = nc.dram_tensor("wp16", [J16, P, HID], fp16, kind="ExternalInput")
    wd16_h = nc.dram_tensor("wd16", [J16, P, HID], fp16, kind="ExternalInput")
    wp8_h = nc.dram_tensor("wp8", [DJ8, P, 2, HID], fp8, kind="ExternalInput")
    wd8_h = nc.dram_tensor("wd8", [DJ8, P, 2, HID], fp8, kind="ExternalInput")
    out_h = nc.dram_tensor("out", [mt_tiles * P, HID], f32, kind="ExternalOutput")

    with tile.TileContext(nc) as tc:
        with (
            tc.tile_pool(name="wpool", bufs=1) as wpool,
            tc.tile_pool(name="xpool", bufs=(2 * xbufs if paired else xbufs)) as xpool,
            tc.tile_pool(name="opool", bufs=obufs) as opool,
            tc.tile_pool(name="psum", bufs=(1 if paired else 2), space="PSUM") as pp,
        ):
            x_tiles = {}

            def load_x(mt):
                tp16 = xpool.tile([P, J16, P], fp16, tag="xp16", name=f"xp16_{mt}")
                nc.sync.dma_start(tp16[:], xp16_h[mt])
                td16 = xpool.tile([P, J16, P], fp16, tag="xd16", name=f"xd16_{mt}")
                nc.sync.dma_start(td16[:], xd16_h[mt])
                tp8 = xpool.tile([P, DJ8, 2, P], fp8, tag="xp8", name=f"xp8_{mt}")
                nc.sync.dma_start(tp8[:], xp8_h[mt])
                td8 = xpool.tile([P, DJ8, 2, P], fp8, tag="xd8", name=f"xd8_{mt}")
                nc.sync.dma_start(td8[:], xd8_h[mt])
                x_tiles[mt] = (tp16, td16, tp8, td8)

            w16_sb = {}
            w8_sb = {}

            def load_weights():
                w16_sb.clear()
                w8_sb.clear()
                for t, h in (("p", wp16_h), ("d", wd16_h)):
                    for j in range(J16):
                        tw = wpool.tile([P, HID], fp16, tag=f"w16{t}{j}",
                                        name=f"w16{t}_{j}")
                        nc.sync.dma_start(tw[:], h[j])
                        w16_sb[t, j] = tw
                for t, h in (("p", wp8_h), ("d", wd8_h)):
                    for dj in range(DJ8):
                        tw = wpool.tile([P, 2, HID], fp8, tag=f"w8{t}{dj}",
                                        name=f"w8{t}_{dj}")
                        nc.sync.dma_start(tw[:], h[dj])
                        w8_sb[t, dj] = tw

            next_load = [1]

            def ensure_loads(upto):
                while next_load[0] <= min(upto, mt_tiles - 1):
                    load_x(next_load[0])
                    next_load[0] += 1

            def m_loop_paired():
                # two m-tiles in flight per j-step (8 PSUM banks)
                assert mt_tiles % 2 == 0
                next_load[0] = 1
                for mt0 in range(0, mt_tiles, 2):
                    pair = [mt0, mt0 + 1]
                    ensure_loads(mt0 + 1 + 2 * (xbufs - 1))
                    xt = {mt: x_tiles.pop(mt) for mt in pair}
                    psums = {
                        (h, nb): pp.tile(
                            [P, NBLK], f32, tag=f"ps{h}_{nb}", name=f"ps_{mt0}_{h}_{nb}"
                        )
                        for h in range(2)
                        for nb in range(NB)
                    }
                    # fp16 strips: j-major, P then D per j
                    for j in range(J16):
                        for ti, t in enumerate(("p", "d")):
                            for h, mt in enumerate(pair):
                                for nb in range(NB):
                                    nc.tensor.matmul(
                                        psums[h, nb][:],
                                        xt[mt][ti][:, j, :],
                                        w16_sb[t, j][:, nb * NBLK:(nb + 1) * NBLK],
                                        start=(j == 0 and ti == 0),
                                        stop=False,
                                    )
                    # fp8 DoubleRow strips: 2 k-strips per matmul
                    for dj in range(DJ8):
                        for ti, t in enumerate(("p", "d")):
                            last = dj == DJ8 - 1 and ti == 1
                            for h, mt in enumerate(pair):
                                for nb in range(NB):
                                    nc.tensor.matmul(
                                        psums[h, nb][:],
                                        xt[mt][2 + ti][:, dj],
                                        w8_sb[t, dj][:, :, nb * NBLK:(nb + 1) * NBLK],
                                        start=False,
                                        stop=last,
                                        perf_mode=DR,
                                    )
                    for h, mt in enumerate(pair):
                        out_t = opool.tile([P, HID], f32, tag="out", name=f"out_{mt}")
                        for nb in range(NB):
                            nc.vector.tensor_scalar_mul(
                                out_t[:, nb * NBLK:(nb + 1) * NBLK],
                                psums[h, nb][:],
                                OSCALE,
                            )
                        nc.sync.dma_start(out_h[mt * P:(mt + 1) * P, :], out_t[:])

            if reps == 1:
                load_x(0)
                load_weights()
                m_loop_paired()
            else:
                with tc.For_i(0, reps, 1):
                    load_x(0)
                    load_weights()
                    m_loop_paired()

    nc.compile()
    return nc


def _q8(a):
    return a.astype(ml_dtypes.float8_e4m3)


def _prep_inputs(protein, drug, Wv_p, Wv_d, mt_tiles=MT_FULL):
    """Host-side shard + transpose-tile + dtype split/cast."""
    kcut = J16 * P

    def prep_w(W):
        W = np.asarray(W, dtype=np.float32) * WSCALE
        w16 = np.ascontiguousarray(W[:kcut].reshape(J16, P, HID).astype(np.float16))
        # w8[dj, p, i, n] = W[kcut + (2dj+i)*P + p, n]
        w8 = W[kcut:].reshape(DJ8, 2, P, HID).transpose(0, 2, 1, 3)
        w8 = np.ascontiguousarray(_q8(w8))
        return w16, w8

    wp16, wp8 = prep_w(Wv_p)
    wd16, wd8 = prep_w(Wv_d)

    def tile_x(x):
        x = x * XSCALE
        # [rows, D] -> fp16 part [mt, p, j, m]: t[mt,p,j,m] = x[mt*P+m, j*P+p]
        t = x.reshape(mt_tiles, P, KT, P).transpose(0, 3, 2, 1)
        t16 = np.ascontiguousarray(t[:, :, :J16, :].astype(np.float16))
        # fp8 part [mt, p, dj, i, m] = x[mt*P+m, (J16+2dj+i)*P+p]
        t8 = t[:, :, J16:, :].reshape(mt_tiles, P, DJ8, 2, P)
        t8 = np.ascontiguousarray(_q8(t8))
        return t16, t8

    protein = np.asarray(protein, dtype=np.float32)
    drug = np.asarray(drug, dtype=np.float32)
    in_maps = []
    rows = mt_tiles * P
    for c in range(NCORES):
        sl = slice(c * M_SH, c * M_SH + rows)
        xp16, xp8 = tile_x(protein[sl])
        xd16, xd8 = tile_x(drug[sl])
        in_maps.append(
            {
                "xp16": xp16, "xd16": xd16, "xp8": xp8, "xd8": xd8,
                "wp16": wp16, "wd16": wd16, "wp8": wp8, "wd8": wd8,
            }
        )
    return in_maps


_MODULE_CACHE = {}


def _run(protein, drug, Wv_p, Wv_d, trace=False, mt_tiles=MT_FULL):
    from concourse.bass_utils import run_bass_kernel_spmd

    nc = _MODULE_CACHE.get(mt_tiles)
    if nc is None:
        nc = _MODULE_CACHE[mt_tiles] = _build_module(mt_tiles)
    in_maps = _prep_inputs(protein, drug, Wv_p, Wv_d, mt_tiles)
    res = run_bass_kernel_spmd(nc, in_maps, list(range(NCORES)), trace=trace)
    E = np.concatenate(
        [np.asarray(r["out"], dtype=np.float32) for r in res.results], axis=0
    )
    return E, res


def kernel(
    protein,
    drug,
    mask_prot=None,
    mask_drug=None,
    Wq_p=None,
    Wk_p=None,
    Wv_p=None,
    Wq_d=None,
    Wk_d=None,
    Wv_d=None,
):
    E, _ = _run(protein, drug, Wv_p, Wv_d, trace=False)
    return np.concatenate([E, E], axis=1)


def kernel_profiled(**inputs):
    E, res = _run(
        inputs["protein"], inputs["drug"], inputs["Wv_p"], inputs["Wv_d"], trace=False
    )
    out = np.concatenate([E, E], axis=1)
    return out, res
